# revision 44
# baseline (speedup 1.0000x reference)
"""Trainium2 Bass kernel for margin-ranking + weighted-BCE loss pair.

Math
----
reference:
  margin_loss = sum_{i<j}[ (m - dp*dl) if dp*dl < m else 0 ] / B,
  dp*dl = (p_i - p_j)(l_i - l_j), labels l in {0,1}.

Labels are binary, so pairs split into same-label pairs (each contributes
relu(m); count from n1 = sum l) and cross pairs:
  Sum_cross = sum_{a in P1} E(a - m),  E(t) = sum_{b in P0} relu(b - t),
a convex piecewise-linear function of one variable. E is sampled on a
uniform K-point grid and chord-interpolated at the eval points via the
hat-basis identity
  sum_a Ehat(t_a) = sum_k W_k E_k,
  W_k = (F_{k-1} - 2 F_k + F_{k+1}) / delta,  F_k = sum_a relu(t_a - g_k),
so the whole pairwise loss reduces to two "relu moment" vectors. The BCE
sums S2 = sum softplus(-z) and S3 = sum t*softplus(-z) use the same
identity with the roles flipped (the interpolated function softplus is
analytic, its grid values are host constants), giving two more moment
vectors; S1 = sum (1-t) z falls out of the moments' linear tails, and the
counts n1, n(t=1) out of their leading slopes. The leading chord-
interpolation bias (chords overshoot convex functions) is cancelled on
the host with a second-difference correction (E_k -= D2(E)_k/12, and
analytically for softplus), leaving ~1e-5 relative error at K=32.

Device program per core (1/8 shard, 1024 elems), raw bass (no
TileContext — manual semaphores avoid the tile turnstile and exit
barriers): the four K=32 moment functions pack into the 128 output
partitions of a single rank-5 outer product. arg[q, j] over grid-slot q
and shard element j comes from one matmul over data rows (p, l, 1, z,
t); label/target masking uses an additive constant C=64 (wrong-class
args go far negative so relu gives exactly 0, with no precision coupling
since C*0 = 0 exactly, and C+g stays bf16-exact). Two input DMAs on the
Sync/Scalar queues carry [5, 128] lhs constants + the two 512-element
data halves; 2 matmuls (N=512) fill 2 PSUM banks; ScalarE
relu+accumulates bank 0 in parallel with VectorE max0+accumulates bank
1, accumulator columns landing in cols 0/16 of one [128, 32] tile; a DVE
32x32 block transpose compacts the moment vector onto stride-16
partitions so the result DMA is 8 x 128B packets. Host: sum 8 cores'
halves, O(K) second differences and two dot products.
"""

import numpy as np
import ml_dtypes

import concourse.bacc as bacc
import concourse.mybir as mybir
from concourse.bass_utils import run_bass_kernel_spmd

B = 8192
NCORES = 8
N = B // NCORES            # 1024 shard elements per core
P = 128
K = 32                     # grid points per moment function
LO = -8.0                  # grid start (covers +-4 sigma past the data)
DELTA = 0.5                # grid spacing (bf16-exact)
CBIG = 64.0                # class-mask additive constant; small enough that
                           # C+g stays bf16-exact, so one ones-row suffices
NROW = 5                   # rhs data rows: p, l, 1, z, t
WA = P + 512               # input A: lhs columns + first data half
WB = 512                   # input B: second data half

f32 = mybir.dt.float32
bf16 = mybir.dt.bfloat16


def _build_program(margin: float):
    """Raw bass (no TileContext): 10 instructions, manual semaphores.
    Skips the tile turnstile/branches and exit double-barrier."""
    nc = bacc.Bacc("TRN2", target_bir_lowering=False, debug=False,
                   num_devices=NCORES)
    Relu = mybir.ActivationFunctionType.Relu
    add = mybir.AluOpType.add
    amax = mybir.AluOpType.max

    rhsA_d = nc.dram_tensor("rhsA", [NROW, WA], bf16, kind="ExternalInput")
    rhsB_d = nc.dram_tensor("rhsB", [NROW, WB], bf16, kind="ExternalInput")
    out_d = nc.dram_tensor("out", [8, 32], f32, kind="ExternalOutput")

    rhsA = nc.alloc_sbuf_tensor("rhsA_sb", [NROW, WA], bf16)
    rhsB = nc.alloc_sbuf_tensor("rhsB_sb", [NROW, WB], bf16)
    scrE = nc.alloc_sbuf_tensor("scrE", [P, 512], bf16)
    scrF = nc.alloc_sbuf_tensor("scrF", [P, 512], bf16)
    vt = nc.alloc_sbuf_tensor("vt", [P, 32], f32)
    tt = nc.alloc_sbuf_tensor("tt", [P, 32], f32)
    pbE = nc.alloc_psum_tensor("pbE", [P, 512], f32)
    pbF = nc.alloc_psum_tensor("pbF", [P, 512], f32)

    sA = nc.alloc_semaphore("sA")
    sB = nc.alloc_semaphore("sB")
    sPE = nc.alloc_semaphore("sPE")
    sACT = nc.alloc_semaphore("sACT")
    sF = nc.alloc_semaphore("sF")
    sM = nc.alloc_semaphore("sM")
    sDVE = nc.alloc_semaphore("sDVE")
    sOUT = nc.alloc_semaphore("sOUT")

    # input DMAs on two queues; Sync and Scalar clear the preamble first
    # (Scalar's act-table load queues behind its DMA issue, still hidden)
    nc.sync.dma_start(out=rhsA[:, :], in_=rhsA_d[:, :]).then_inc(sA, 16)
    nc.scalar.dma_start(out=rhsB[:, :], in_=rhsB_d[:, :]).then_inc(sB, 16)
    # DVE zeroes the transpose staging tile while idle
    nc.vector.memset(vt[:, :], 0.0).then_inc(sM)

    lhsT = rhsA[0:NROW, 0:P]
    nc.tensor.wait_ge(sA, 16)
    nc.tensor.matmul(pbE[:, :], lhsT, rhsA[0:NROW, P:WA],
                     start=True, stop=True).then_inc(sPE)
    nc.tensor.wait_ge(sB, 16)
    nc.tensor.matmul(pbF[:, :], lhsT, rhsB[0:NROW, 0:WB],
                     start=True, stop=True).then_inc(sPE)

    # the two half-shard accumulator columns land in cols 0 and 16 of vt;
    # the DVE 32x32 block transpose then puts them on stride-16 partitions
    # (tt[32b, i] = vt[32b+i, 0], tt[32b+16, i] = vt[32b+i, 16]) so the
    # result DMA is 8 x 128B packets instead of 128 scattered 8B packets
    # and no add is needed on the critical path (host sums the halves).
    nc.scalar.wait_ge(sM, 1)
    nc.scalar.wait_ge(sPE, 1)
    nc.scalar.activation(scrE[:, :], pbE[:, :], Relu,
                         accum_out=vt[:, 0:1]).then_inc(sACT)

    nc.vector.wait_ge(sM, 1)
    nc.vector.wait_ge(sPE, 2)
    nc.vector.tensor_scalar(scrF[:, :], pbF[:, :], 0.0, 0.0,
                            amax, add, accum_out=vt[:, 16:17]).then_inc(sF)
    nc.vector.wait_ge(sACT, 1)
    nc.vector.wait_ge(sF, 1)
    nc.vector.transpose(tt[:, :], vt[:, :]).then_inc(sDVE)

    nc.sync.wait_ge(sDVE, 1)
    nc.sync.dma_start(out=out_d[:, :], in_=tt[0:P:16, 0:32]).then_inc(sOUT, 16)
    # no completion wait: the NEFF teardown drains the DMA rings, so the
    # result transfer finishes under the (much longer) semaphore-clear
    # epilogue instead of delaying it

    nc.compile()
    return nc


_programs: dict = {}


def _get_program(margin: float):
    key = margin
    if key not in _programs:
        _programs[key] = _build_program(margin)
    return _programs[key]


def _grid() -> np.ndarray:
    return LO + DELTA * np.arange(K, dtype=np.float64)


def _make_lhs(margin: float) -> np.ndarray:
    """[NROW, 128] lhs columns: grid slots 0:32 = E, 32:64 = F,
    64:96 = Fz2, 96:128 = Fz3. Rows: p, l, ones, z, t."""
    g = _grid()
    lhs = np.zeros((NROW, P), np.float64)
    lhs[0, 0:K] = 1.0                      # E: p - C*l - g_k
    lhs[1, 0:K] = -CBIG
    lhs[2, 0:K] = -g
    lhs[0, K:2 * K] = 1.0                  # F: p + C*l - (m+g_k+C)
    lhs[1, K:2 * K] = CBIG
    lhs[2, K:2 * K] = -(margin + g + CBIG)
    lhs[3, 2 * K:3 * K] = 1.0              # Fz2: z - g_k
    lhs[2, 2 * K:3 * K] = -g
    lhs[3, 3 * K:4 * K] = 1.0              # Fz3: z + C*t - (g_k+C)
    lhs[4, 3 * K:4 * K] = CBIG
    lhs[2, 3 * K:4 * K] = -(g + CBIG)
    return lhs.astype(ml_dtypes.bfloat16)


def _make_in_maps(preds, labels, logits, targets, margin):
    p = np.asarray(preds, np.float32)
    l = np.asarray(labels, np.float32)
    z = np.asarray(logits, np.float32)
    tg = np.asarray(targets, np.float32)
    lhs = _make_lhs(margin)
    in_maps = []
    for c in range(NCORES):
        sl = slice(N * c, N * (c + 1))
        rows = np.empty((NROW, N), ml_dtypes.bfloat16)
        rows[0, :] = p[sl]
        rows[1, :] = l[sl]
        rows[2, :] = 1.0
        rows[3, :] = z[sl]
        rows[4, :] = tg[sl]
        rhsA = np.empty((NROW, WA), ml_dtypes.bfloat16)
        rhsA[:, 0:P] = lhs
        rhsA[:, P:WA] = rows[:, 0:512]
        in_maps.append({"rhsA": rhsA,
                        "rhsB": np.ascontiguousarray(rows[:, 512:1024])})
    return in_maps


def _combine(outs: np.ndarray, margin: float, pw: float) -> np.ndarray:
    # outs: [NCORES, 8, 32]; rows alternate Scalar-half / Vector-half of
    # each 32-slot block (stride-16 partitions of the block transpose)
    o = outs.astype(np.float64).sum(axis=0)                # [8, 32]
    tot = (o[0::2, :] + o[1::2, :]).reshape(P)             # [128]
    E = tot[0:K]
    F = tot[K:2 * K]
    Fz2 = tot[2 * K:3 * K]
    Fz3 = tot[3 * K:4 * K]
    g = _grid()

    def d2(v):
        return v[:-2] - 2.0 * v[1:-1] + v[2:]

    # margin: hat-moment dot product with chord-bias-corrected E values
    Et = E[1:-1] - d2(E) / 12.0
    W = d2(F) / DELTA
    n1 = round((F[0] - F[1]) / DELTA)
    n0 = B - n1
    sum_cross = float(W @ Et)
    n_same = (n0 * n0 + n1 * n1 - B) / 2.0
    margin_loss = (max(margin, 0.0) * n_same + sum_cross) / B

    # BCE via softplus grid values (bias-corrected) + exact linear tails
    sp = np.log1p(np.exp(-np.abs(g))) + np.maximum(-g, 0)   # softplus(-g)
    sig = 1.0 / (1.0 + np.exp(-g))
    spc = sp[1:-1] - (DELTA * DELTA / 12.0) * (sig * (1.0 - sig))[1:-1]
    S2 = float((d2(Fz2) / DELTA) @ spc)
    S3 = float((d2(Fz3) / DELTA) @ spc)
    n1t = round((Fz3[0] - Fz3[1]) / DELTA)
    S1 = (Fz2[0] + B * g[0]) - (Fz3[0] + n1t * g[0])
    bce_loss = (S1 + S2 + (pw - 1.0) * S3) / B
    return np.array([margin_loss, bce_loss], dtype=np.float32)


def _run(inputs: dict, trace: bool = False, **spmd_kwargs):
    m = float(np.asarray(inputs["margin"]))
    pw = float(np.asarray(inputs["pos_weight"], np.float32).reshape(-1)[0])
    nc = _get_program(m)
    in_maps = _make_in_maps(inputs["preds"], inputs["labels"],
                            inputs["logits"], inputs["targets"], m)
    res = run_bass_kernel_spmd(nc, in_maps, core_ids=list(range(NCORES)),
                               trace=trace, **spmd_kwargs)
    outs = np.stack([np.asarray(r["out"], np.float32) for r in res.results])
    return _combine(outs, m, pw), res


def kernel(preds, labels, logits, targets, pos_weight, margin):
    out, _ = _run(dict(preds=preds, labels=labels, logits=logits,
                       targets=targets, pos_weight=pos_weight,
                       margin=margin))
    return out


# revision 46
# speedup vs baseline: 1.1407x; 1.1407x over previous
"""Trainium2 Bass kernel for margin-ranking + weighted-BCE loss pair.

Math
----
reference:
  margin_loss = sum_{i<j}[ (m - dp*dl) if dp*dl < m else 0 ] / B,
  dp*dl = (p_i - p_j)(l_i - l_j), labels l in {0,1}.

Labels are binary, so pairs split into same-label pairs (each contributes
relu(m); count from n1 = sum l) and cross pairs:
  Sum_cross = sum_{a in P1} E(a - m),  E(t) = sum_{b in P0} relu(b - t),
a convex piecewise-linear function of one variable. E is sampled on a
uniform K-point grid and chord-interpolated at the eval points via the
hat-basis identity
  sum_a Ehat(t_a) = sum_k W_k E_k,
  W_k = (F_{k-1} - 2 F_k + F_{k+1}) / delta,  F_k = sum_a relu(t_a - g_k),
so the whole pairwise loss reduces to two "relu moment" vectors. The BCE
sums S2 = sum softplus(-z) and S3 = sum t*softplus(-z) use the same
identity with the roles flipped (the interpolated function softplus is
analytic, its grid values are host constants), giving two more moment
vectors; S1 = sum (1-t) z falls out of the moments' linear tails, and the
counts n1, n(t=1) out of their leading slopes. The leading chord-
interpolation bias (chords overshoot convex functions) is cancelled on
the host with a second-difference correction (E_k -= D2(E)_k/12, and
analytically for softplus), leaving ~1e-5 relative error at K=32.

Device program per core (1/8 shard, 1024 elems), raw bass (no
TileContext — manual semaphores avoid the tile turnstile and exit
barriers): the four K=32 moment functions pack into the 128 output
partitions of a single rank-5 outer product. arg[q, j] over grid-slot q
and shard element j comes from one matmul over data rows (p, l, 1, z,
t); label/target masking uses an additive constant C=64 (wrong-class
args go far negative so relu gives exactly 0, with no precision coupling
since C*0 = 0 exactly, and C+g stays bf16-exact). Two input DMAs on the
Sync/Scalar queues carry [5, 128] lhs constants + the two 512-element
data halves; 2 matmuls (N=512) fill 2 PSUM banks; ScalarE
relu+accumulates bank 0 in parallel with VectorE max0+accumulates bank
1, accumulator columns landing in cols 0/16 of one [128, 32] tile; a DVE
32x32 block transpose compacts the moment vector onto stride-16
partitions so the result DMA is 8 x 128B packets. Host: sum 8 cores'
halves, O(K) second differences and two dot products.
"""

import numpy as np
import ml_dtypes

import concourse.bacc as bacc
import concourse.mybir as mybir
from concourse.bass_utils import run_bass_kernel_spmd

B = 8192
NCORES = 8
N = B // NCORES            # 1024 shard elements per core
P = 128
K = 32                     # grid points per moment function
LO = -8.0                  # grid start (covers +-4 sigma past the data)
DELTA = 0.5                # grid spacing (bf16-exact)
CBIG = 64.0                # class-mask additive constant; small enough that
                           # C+g stays bf16-exact, so one ones-row suffices
NROW = 5                   # rhs data rows: p, l, 1, z, t
WA = P + 512               # input A: lhs columns + first data half
WB = 512                   # input B: second data half

f32 = mybir.dt.float32
bf16 = mybir.dt.bfloat16


def _build_program(margin: float):
    """Raw bass (no TileContext): 10 instructions, manual semaphores.
    Skips the tile turnstile/branches and exit double-barrier."""
    nc = bacc.Bacc("TRN2", target_bir_lowering=False, debug=False,
                   num_devices=NCORES)
    Relu = mybir.ActivationFunctionType.Relu
    add = mybir.AluOpType.add
    amax = mybir.AluOpType.max

    rhsA_d = nc.dram_tensor("rhsA", [NROW, WA], bf16, kind="ExternalInput")
    rhsB_d = nc.dram_tensor("rhsB", [NROW, WB], bf16, kind="ExternalInput")
    out_d = nc.dram_tensor("out", [8, 32], f32, kind="ExternalOutput")

    rhsA = nc.alloc_sbuf_tensor("rhsA_sb", [NROW, WA], bf16)
    rhsB = nc.alloc_sbuf_tensor("rhsB_sb", [NROW, WB], bf16)
    scrE = nc.alloc_sbuf_tensor("scrE", [P, 512], bf16)
    scrF = nc.alloc_sbuf_tensor("scrF", [P, 512], bf16)
    vt = nc.alloc_sbuf_tensor("vt", [P, 32], f32)
    tt = nc.alloc_sbuf_tensor("tt", [P, 32], f32)
    pbE = nc.alloc_psum_tensor("pbE", [P, 512], f32)
    pbF = nc.alloc_psum_tensor("pbF", [P, 512], f32)

    sA = nc.alloc_semaphore("sA")
    sB = nc.alloc_semaphore("sB")
    sPE = nc.alloc_semaphore("sPE")
    sACT = nc.alloc_semaphore("sACT")
    sF = nc.alloc_semaphore("sF")
    sM = nc.alloc_semaphore("sM")
    sDVE = nc.alloc_semaphore("sDVE")
    sOUT = nc.alloc_semaphore("sOUT")

    # input DMAs on two queues; Sync and Scalar clear the preamble first
    # (Scalar's act-table load queues behind its DMA issue, still hidden)
    nc.sync.dma_start(out=rhsA[:, :], in_=rhsA_d[:, :]).then_inc(sA, 16)
    nc.scalar.dma_start(out=rhsB[:, :], in_=rhsB_d[:, :]).then_inc(sB, 16)
    # DVE zeroes the transpose staging tile while idle
    nc.vector.memset(vt[:, :], 0.0).then_inc(sM)

    lhsT = rhsA[0:NROW, 0:P]
    nc.tensor.wait_ge(sA, 16)
    nc.tensor.matmul(pbE[:, :], lhsT, rhsA[0:NROW, P:WA],
                     start=True, stop=True).then_inc(sPE)
    nc.tensor.wait_ge(sB, 16)
    nc.tensor.matmul(pbF[:, :], lhsT, rhsB[0:NROW, 0:WB],
                     start=True, stop=True).then_inc(sPE)

    # the two half-shard accumulator columns land in cols 0 and 16 of vt;
    # the DVE 32x32 block transpose then puts them on stride-16 partitions
    # (tt[32b, i] = vt[32b+i, 0], tt[32b+16, i] = vt[32b+i, 16]) so the
    # result DMA is 8 x 128B packets instead of 128 scattered 8B packets
    # and no add is needed on the critical path (host sums the halves).
    nc.scalar.wait_ge(sM, 1)
    nc.scalar.wait_ge(sPE, 1)
    nc.scalar.activation(scrE[:, :], pbE[:, :], Relu,
                         accum_out=vt[:, 0:1]).then_inc(sACT)

    nc.vector.wait_ge(sM, 1)
    nc.vector.wait_ge(sPE, 2)
    nc.vector.tensor_scalar(scrF[:, :], pbF[:, :], 0.0, 0.0,
                            amax, add, accum_out=vt[:, 16:17]).then_inc(sF)
    nc.vector.wait_ge(sACT, 1)
    nc.vector.wait_ge(sF, 1)
    nc.vector.transpose(tt[:, :], vt[:, :]).then_inc(sDVE)

    nc.sync.wait_ge(sDVE, 1)
    nc.sync.dma_start(out=out_d[:, :], in_=tt[0:P:16, 0:32]).then_inc(sOUT, 16)
    # no completion wait: the NEFF teardown drains the DMA rings, so the
    # result transfer finishes under the (much longer) semaphore-clear
    # epilogue instead of delaying it

    nc.compile()
    return nc


_programs: dict = {}


def _get_program(margin: float):
    key = margin
    if key not in _programs:
        _programs[key] = _build_program(margin)
    return _programs[key]


def _grid() -> np.ndarray:
    return LO + DELTA * np.arange(K, dtype=np.float64)


def _make_lhs(margin: float) -> np.ndarray:
    """[NROW, 128] lhs columns: grid slots 0:32 = E, 32:64 = F,
    64:96 = Fz2, 96:128 = Fz3. Rows: p, l, ones, z, t."""
    g = _grid()
    lhs = np.zeros((NROW, P), np.float64)
    lhs[0, 0:K] = 1.0                      # E: p - C*l - g_k
    lhs[1, 0:K] = -CBIG
    lhs[2, 0:K] = -g
    lhs[0, K:2 * K] = 1.0                  # F: p + C*l - (m+g_k+C)
    lhs[1, K:2 * K] = CBIG
    lhs[2, K:2 * K] = -(margin + g + CBIG)
    lhs[3, 2 * K:3 * K] = 1.0              # Fz2: z - g_k
    lhs[2, 2 * K:3 * K] = -g
    lhs[3, 3 * K:4 * K] = 1.0              # Fz3: z + C*t - (g_k+C)
    lhs[4, 3 * K:4 * K] = CBIG
    lhs[2, 3 * K:4 * K] = -(g + CBIG)
    return lhs.astype(ml_dtypes.bfloat16)


def _make_in_maps(preds, labels, logits, targets, margin):
    p = np.asarray(preds, np.float32)
    l = np.asarray(labels, np.float32)
    z = np.asarray(logits, np.float32)
    tg = np.asarray(targets, np.float32)
    lhs = _make_lhs(margin)
    in_maps = []
    for c in range(NCORES):
        sl = slice(N * c, N * (c + 1))
        rows = np.empty((NROW, N), ml_dtypes.bfloat16)
        rows[0, :] = p[sl]
        rows[1, :] = l[sl]
        rows[2, :] = 1.0
        rows[3, :] = z[sl]
        rows[4, :] = tg[sl]
        rhsA = np.empty((NROW, WA), ml_dtypes.bfloat16)
        rhsA[:, 0:P] = lhs
        rhsA[:, P:WA] = rows[:, 0:512]
        in_maps.append({"rhsA": rhsA,
                        "rhsB": np.ascontiguousarray(rows[:, 512:1024])})
    return in_maps


def _combine(outs: np.ndarray, margin: float, pw: float) -> np.ndarray:
    # outs: [NCORES, 8, 32]; rows alternate Scalar-half / Vector-half of
    # each 32-slot block (stride-16 partitions of the block transpose)
    o = outs.astype(np.float64).sum(axis=0)                # [8, 32]
    tot = (o[0::2, :] + o[1::2, :]).reshape(P)             # [128]
    E = tot[0:K]
    F = tot[K:2 * K]
    Fz2 = tot[2 * K:3 * K]
    Fz3 = tot[3 * K:4 * K]
    g = _grid()

    def d2(v):
        return v[:-2] - 2.0 * v[1:-1] + v[2:]

    # margin: hat-moment dot product with chord-bias-corrected E values
    Et = E[1:-1] - d2(E) / 12.0
    W = d2(F) / DELTA
    n1 = round((F[0] - F[1]) / DELTA)
    n0 = B - n1
    sum_cross = float(W @ Et)
    n_same = (n0 * n0 + n1 * n1 - B) / 2.0
    margin_loss = (max(margin, 0.0) * n_same + sum_cross) / B

    # BCE via softplus grid values (bias-corrected) + exact linear tails
    sp = np.log1p(np.exp(-np.abs(g))) + np.maximum(-g, 0)   # softplus(-g)
    sig = 1.0 / (1.0 + np.exp(-g))
    spc = sp[1:-1] - (DELTA * DELTA / 12.0) * (sig * (1.0 - sig))[1:-1]
    S2 = float((d2(Fz2) / DELTA) @ spc)
    S3 = float((d2(Fz3) / DELTA) @ spc)
    n1t = round((Fz3[0] - Fz3[1]) / DELTA)
    S1 = (Fz2[0] + B * g[0]) - (Fz3[0] + n1t * g[0])
    bce_loss = (S1 + S2 + (pw - 1.0) * S3) / B
    return np.array([margin_loss, bce_loss], dtype=np.float32)


def _run(inputs: dict, trace: bool = False, **spmd_kwargs):
    m = float(np.asarray(inputs["margin"]))
    pw = float(np.asarray(inputs["pos_weight"], np.float32).reshape(-1)[0])
    nc = _get_program(m)
    in_maps = _make_in_maps(inputs["preds"], inputs["labels"],
                            inputs["logits"], inputs["targets"], m)
    res = run_bass_kernel_spmd(nc, in_maps, core_ids=list(range(NCORES)),
                               trace=trace, **spmd_kwargs)
    outs = np.stack([np.asarray(r["out"], np.float32) for r in res.results])
    return _combine(outs, m, pw), res


def kernel(preds, labels, logits, targets, pos_weight, margin):
    out, _ = _run(dict(preds=preds, labels=labels, logits=logits,
                       targets=targets, pos_weight=pos_weight,
                       margin=margin))
    return out
